# revision 2
# baseline (speedup 1.0000x reference)
"""CapsuleLayer (dynamic routing) Trainium2 kernel.

x: [128, 2048, 8] f32, W: [2048, 32, 8, 16] f32 -> v: [128, 32, 16] f32

Sharding: batch B=128 split across 8 cores (16 each), W replicated (96 of
128 j2-tiles resident in SBUF bf16, rest streamed).  Per core, per routing
pass, u_hat tiles ([128, 1024] = 16 caps x 16 batch x 512 (o,d)) are
recomputed on the PE via a block-diagonal-x matmul and consumed on-chip.

Engine split per j2: PE u-matmuls + softmax-weighted n-reduction (matmul
whose stationary selrz = block-ones * 1/Z folds the softmax normalize),
ACT PSUM->SBUF cast + exp, DVE q=u*v and a 2x add-tree for the d-sum
(TensorReduce is 1x-only), Pool most e = u*expb multiplies.  Emission is
software-pipelined: produce(group g+1) interleaves with consume(group g)
at j2 granularity so PE's in-order queue never head-of-line blocks the
next group's casts.
"""

from contextlib import ExitStack

import numpy as np
import ml_dtypes

import concourse.bass as bass
import concourse.bacc as bacc
import concourse.tile as tile
from concourse import mybir
from concourse.bass_utils import run_bass_kernel_spmd

BF16 = mybir.dt.bfloat16
F32 = mybir.dt.float32
X = mybir.AxisListType.X
Exp = mybir.ActivationFunctionType.Exp
Copy = mybir.ActivationFunctionType.Copy

B, N, O, I, D = 128, 2048, 32, 8, 16
CORES = 8
BL = B // CORES            # 16 batch elements per core
J2 = N // 16               # 128 blocks of 16 input caps
OD = O * D                 # 512
G = 4                      # j2 group size for batched softmax
NG = J2 // G               # groups per pass
JRES = 96                  # bf16 W j2-tiles resident in SBUF

_BF = ml_dtypes.bfloat16


def _bcast_last(ap, count):
    """Append a step-0 (broadcast) innermost dim to an AP."""
    return bass.AP(tensor=ap.tensor, offset=ap.offset, ap=list(ap.ap) + [[0, count]])


def build_nc():
    nc = bacc.Bacc("TRN2", target_bir_lowering=False)

    w = nc.dram_tensor("w", [128, J2, OD], BF16, kind="ExternalInput")
    xt = nc.dram_tensor("xt", [128, J2, BL], BF16, kind="ExternalInput")
    xbd = nc.dram_tensor("xbd", [J2, 128, 2 * 128], BF16, kind="ExternalInput")
    ones = nc.dram_tensor("ones", [128, 8], BF16, kind="ExternalInput")
    sel16 = nc.dram_tensor("sel16", [16, 2, 128], BF16, kind="ExternalInput")
    out = nc.dram_tensor("out", [BL, OD], F32, kind="ExternalOutput")

    with tile.TileContext(nc) as tc, ExitStack() as ctx:
        xbdp = ctx.enter_context(tc.tile_pool(name="xbdp", bufs=8))
        wsp = ctx.enter_context(tc.tile_pool(name="wsp", bufs=4))
        const = ctx.enter_context(tc.tile_pool(name="const", bufs=1))
        biasp = ctx.enter_context(tc.tile_pool(name="biasp", bufs=1))
        vexpp = ctx.enter_context(tc.tile_pool(name="vexpp", bufs=2))
        work = ctx.enter_context(tc.tile_pool(name="work", bufs=3))
        small = ctx.enter_context(tc.tile_pool(name="small", bufs=6))
        sqp = ctx.enter_context(tc.tile_pool(name="sqp", bufs=1))
        psum_u = ctx.enter_context(tc.tile_pool(name="psum_u", bufs=3, space="PSUM"))
        psum_s = ctx.enter_context(tc.tile_pool(name="psum_s", bufs=1, space="PSUM"))

        Mult = mybir.AluOpType.mult

        ones_sb = const.tile([128, 8], BF16)
        nc.sync.dma_start(out=ones_sb[:], in_=ones[:])
        sel_sb = const.tile([16, 2, 128], BF16)
        nc.sync.dma_start(out=sel_sb[:], in_=sel16[:])
        xt_all = const.tile([128, J2, BL], BF16)
        nc.sync.dma_start(out=xt_all[:], in_=xt[:])
        w_all = const.tile([128, JRES, OD], BF16)
        for ch in range(6):
            nc.sync.dma_start(
                out=w_all[:, ch * 16 : (ch + 1) * 16, :],
                in_=w[:][:, ch * 16 : (ch + 1) * 16, :],
            )

        def w_tile(j2):
            if j2 < JRES:
                return w_all[:, j2, :]
            wt = wsp.tile([128, OD], BF16, tag="wst")
            nc.sync.dma_start(out=wt[:], in_=w[:][:, j2, :])
            return wt[:]

        # bias logits [(n16 b8) partition, (j2, h, o)] bf16
        bias_all = biasp.tile([128, J2, 2, O], BF16)

        # prewarm ACT sqrt/exp tables so LoadActFuncSet is off the critical path
        warm = sqp.tile([1, 2], F32, tag="warm")
        nc.vector.memset(warm[:], 1.0)
        nc.scalar.sqrt(warm[:, 0:1], warm[:, 0:1])
        nc.scalar.activation(warm[:, 1:2], warm[:, 0:1], Exp)

        epsb = const.tile([128, 1], F32)
        nc.vector.memset(epsb[:], 1e-8)

        def squash(s_ap, P, v_ap):
            """v = s * |s|^2/(1+|s|^2) / sqrt(|s|^2 + 1e-8), per (b, o) over d."""
            s_sb = sqp.tile([P, OD], F32, tag="s_sb")
            nc.scalar.activation(s_sb[:], s_ap, Copy)
            ssq = sqp.tile([P, OD], F32, tag="ssq")
            nc.vector.tensor_mul(ssq[:], s_sb[:], s_sb[:])
            sq = sqp.tile([P, O], F32, tag="sq")
            nc.vector.reduce_sum(
                out=sq[:], in_=ssq[:].rearrange("p (o d) -> p o d", d=D), axis=X
            )
            rt = sqp.tile([P, O], F32, tag="rt")
            nc.scalar.activation(rt[:], sq[:], mybir.ActivationFunctionType.Sqrt, bias=epsb[:P, :])
            g = sqp.tile([P, O], F32, tag="g")
            nc.vector.scalar_tensor_tensor(
                g[:], sq[:], 1.0, rt[:], mybir.AluOpType.add, Mult
            )
            rg = sqp.tile([P, O], F32, tag="rg")
            nc.vector.reciprocal(rg[:], g[:])
            scale = sqp.tile([P, O], F32, tag="scale")
            nc.vector.tensor_mul(scale[:], sq[:], rg[:])
            nc.vector.tensor_mul(
                v_ap.rearrange("p (o d) -> p o d", d=D),
                s_sb[:].rearrange("p (o d) -> p o d", d=D),
                _bcast_last(scale[:], D),
            )

        def squash2(s_ap, P, v_ap):
            """squash() over [P, 2*OD] treating (h,o) as 64 capsules."""
            s_sb = sqp.tile([P, 2 * OD], F32, tag="s2_sb")
            nc.scalar.activation(s_sb[:], s_ap, Copy)
            ssq = sqp.tile([P, 2 * OD], F32, tag="s2sq")
            nc.vector.tensor_mul(ssq[:], s_sb[:], s_sb[:])
            sq = sqp.tile([P, 2 * O], F32, tag="s2q")
            nc.vector.reduce_sum(
                out=sq[:], in_=ssq[:].rearrange("p (o d) -> p o d", d=D), axis=X
            )
            rt = sqp.tile([P, 2 * O], F32, tag="s2rt")
            nc.scalar.activation(
                rt[:], sq[:], mybir.ActivationFunctionType.Sqrt, bias=epsb[:P, :]
            )
            g = sqp.tile([P, 2 * O], F32, tag="s2g")
            nc.vector.scalar_tensor_tensor(
                g[:], sq[:], 1.0, rt[:], mybir.AluOpType.add, Mult
            )
            rg = sqp.tile([P, 2 * O], F32, tag="s2rg")
            nc.vector.reciprocal(rg[:], g[:])
            scale = sqp.tile([P, 2 * O], F32, tag="s2scale")
            nc.vector.tensor_mul(scale[:], sq[:], rg[:])
            nc.vector.tensor_mul(
                v_ap.rearrange("p (o d) -> p o d", d=D),
                s_sb[:].rearrange("p (o d) -> p o d", d=D),
                _bcast_last(scale[:], D),
            )

        def make_vexp(vfull):
            """vfull: [16, OD] bf16 tile (v rows) -> vexp [128, 2*OD] tile.

            vexp[p=(n16 b8), h*OD + (o,d)] = v[h*8 + p%8, o, d], built with two
            selector matmuls (sel16[k,h,m] = d(k, h*8+m%8)) + one ACT cast, so
            no SP-sequencer DMA sits on the pass-boundary critical path.
            """
            vx_ps = psum_u.tile([128, 2 * OD], F32, tag="ups")
            for h in range(2):
                nc.tensor.matmul(
                    vx_ps[:, h * OD : (h + 1) * OD],
                    sel_sb[:, h, :],
                    vfull[:],
                    start=True,
                    stop=True,
                )
            vx = vexpp.tile([128, 2 * OD], BF16, tag="vexp")
            nc.scalar.activation(vx[:], vx_ps[:], Copy)
            return vx

        def produce_u(k, j2):
            xbd_t = xbdp.tile([128, 2 * 128], BF16)
            nc.sync.dma_start(out=xbd_t[:], in_=xbd[:][j2, :, :])
            u_ps = psum_u.tile([128, 2 * OD], F32, tag="ups")
            wj = w_tile(j2)
            for h in range(2):
                nc.tensor.matmul(
                    u_ps[:, h * OD : (h + 1) * OD],
                    xbd_t[:, h * 128 : (h + 1) * 128],
                    wj,
                    start=True,
                    stop=True,
                )
            u_sb = work.tile([128, 2 * OD], BF16, tag="usb", bufs=16)
            nc.scalar.activation(u_sb[:], u_ps[:], Copy)
            return u_sb

        # ---------------- pass 1 (iter 0): s0 = sum_n u_hat / 32 ----------------
        s0_t = psum_s.tile([BL, 2 * OD], F32, tag="sacc")
        s0_ps = s0_t[:, :OD]
        # Interleave the first pass-2 u-productions into the s0 chain: the
        # s0 matmuls are gated on W-chunk DMA arrival, so PE/ACT have slack
        # to pre-produce u tiles; Pool then has a deep backlog at v0-time.
        NPREF = 8
        prefix_usb = {}
        for j2 in range(J2):
            nc.tensor.matmul(
                s0_ps,
                xt_all[:, j2, :],
                w_tile(j2),
                start=(j2 == 0),
                stop=(j2 == J2 - 1),
            )
            if j2 % 8 == 7 and len(prefix_usb) < NPREF:
                pj = len(prefix_usb)
                prefix_usb[pj] = produce_u(1, pj)
        vexp = [None]

        def end_of_pass0():
            v_full1 = vexpp.tile([BL, OD], BF16, tag="vfull")
            squash(s0_ps, BL, v_full1[:])
            vexp[0] = make_vexp(v_full1)

        # ---------------- passes 2, 3 (iters 1, 2): software pipeline ----------
        # produce_u(k, j2): xbd DMA + 2 u-matmuls + ACT cast        (no v dep)
        # produce_q(k, j2): q = u*vexp, 2x add-tree -> bias logits  (needs vexp)
        # smalls(k, jg):    exp, Z-reduce, 1/Z, selrz               (needs trees)
        # consume(k, jg, jj): e = u*expb (Pool/DVE), s-matmuls      (needs smalls)

        s_ps = {}
        pend_badd = []

        def produce_q(k, j2, u_sb, ex, se):
            q = work.tile([128, 2 * OD], BF16, tag="q", bufs=2)
            nc.vector.tensor_mul(q[:], u_sb[:], vexp[0][:])
            qv = q[:].rearrange("p (ho d) -> p ho d", d=D)
            t1 = work.tile([128, 64, 8], BF16, tag="t1", bufs=1)
            nc.vector.tensor_add(t1[:], qv[:, :, 0:8], qv[:, :, 8:16])
            t2 = work.tile([128, 64, 4], BF16, tag="t2", bufs=1)
            nc.vector.tensor_add(t2[:], t1[:, :, 0:4], t1[:, :, 4:8])
            t3 = work.tile([128, 64, 2], BF16, tag="t3", bufs=1)
            nc.vector.tensor_add(t3[:], t2[:, :, 0:2], t2[:, :, 2:4])
            bias_slice = bias_all[:, j2, :, :]  # [128, 2, O]
            if k == 1:
                nc.vector.tensor_add(
                    bias_slice.rearrange("p h o -> p (h o)"),
                    t3[:, :, 0],
                    t3[:, :, 1],
                )
            else:
                a2 = small.tile([128, 64], BF16, name="a2", tag="a2")
                nc.vector.tensor_add(a2[:], t3[:, :, 0], t3[:, :, 1])
                nc.vector.tensor_add(
                    bias_slice.rearrange("p h o -> p (h o)"),
                    bias_slice.rearrange("p h o -> p (h o)"),
                    a2[:],
                )
            jj = j2 % G
            for h in range(2):
                nc.scalar.activation(
                    ex[:, jj, h, :],
                    bias_slice[:, h, :],
                    Exp,
                    accum_out=se[:, 2 * jj + h : 2 * jj + h + 1],
                )


        def smalls(k, jg, ex, se):
            rse = small.tile([128, G * 2], F32, tag="rse")
            nc.vector.reciprocal(rse[:], se[:])
            rse = rse[:].rearrange("p (g h) -> p g h", h=2)
            selrz = small.tile([128, G, 2, 8], BF16, tag="selrz")
            oap = ones_sb[:]
            ones_b = bass.AP(
                tensor=oap.tensor,
                offset=oap.offset,
                ap=[list(oap.ap)[0], [0, G], [0, 2], list(oap.ap)[1]],
            )
            nc.vector.tensor_mul(selrz[:], ones_b, _bcast_last(rse, 8))
            return selrz

        def consume(k, jg, jj, u_sb, ex, selrz, n_dve):
            j2 = jg * G + jj
            e_t = work.tile([128, 2 * OD], BF16, tag="et", bufs=6)
            for h in range(2):
                eng = nc.vector if (2 * jj + h) >= 8 - n_dve else nc.gpsimd
                eng.tensor_mul(
                    e_t[:, h * OD : (h + 1) * OD].rearrange("p (o d) -> p o d", d=D),
                    u_sb[:, h * OD : (h + 1) * OD].rearrange("p (o d) -> p o d", d=D),
                    _bcast_last(ex[:, jj, h, :], D),
                )
            for h in range(2):
                nc.tensor.matmul(
                    s_ps[k][:, h * OD : (h + 1) * OD],
                    selrz[:, jj, h, :],
                    e_t[:, h * OD : (h + 1) * OD],
                    start=(j2 == 0),
                    stop=(j2 == J2 - 1),
                )

        def make_vexp8(vtmp):
            """vtmp: [8, 2*OD] bf16 (v rows as h-halves) -> vexp tile.

            vexp[p, h*OD+(o,d)] = vtmp[p%8, h*OD+(o,d)] via K=8 selector
            matmuls; no staging DMA on the pass-boundary critical path.
            """
            vx_ps = psum_u.tile([128, 2 * OD], F32, tag="ups")
            sel8 = sel_sb[0:8, 0, :]
            for h in range(2):
                nc.tensor.matmul(
                    vx_ps[:, h * OD : (h + 1) * OD],
                    sel8,
                    vtmp[:, h * OD : (h + 1) * OD],
                    start=True,
                    stop=True,
                )
            vx = vexpp.tile([128, 2 * OD], BF16, tag="vexp")
            nc.scalar.activation(vx[:], vx_ps[:], Copy)
            return vx

        def end_of_pass(k):
            """squash(s) -> v; rebuild vexp (k<2) or write output (k=2)."""
            sp = s_ps[k]
            if k == 1:
                vtmp = sqp.tile([8, 2 * OD], BF16, tag="vtmp")
                squash2(sp, 8, vtmp[:])
                vexp[0] = make_vexp8(vtmp)
            else:
                v_f32 = sqp.tile([8, 2 * OD], F32, tag="vf32")
                squash2(sp, 8, v_f32[:])
                for h in range(2):
                    nc.sync.dma_start(
                        out=out[:][h * 8 : (h + 1) * 8, :],
                        in_=v_f32[:, h * OD : (h + 1) * OD],
                    )

        # DVE e-share pattern: ~1.5 of 8 e-halves per group go to DVE
        def dve_halves(k, jg):
            return 2

        groups = [(k, jg) for k in (1, 2) for jg in range(NG)]
        DEPTH = 2
        pipeline = []  # entries: [k, jg, usb, ex, se, selrz_holder]
        pend_q = []    # produce_q deferred until the pass's vexp exists
        pend_sm = []   # entries with smalls deferred for the same reason
        vexp_ready = {1: False, 2: False}

        def flush_pend():
            for qk, qj2, qusb, qex, qse in pend_q:
                produce_q(qk, qj2, qusb, qex, qse)
            pend_q.clear()
            for ent in pend_sm:
                ent[5][0] = smalls(ent[0], ent[1], ent[3], ent[4])
            pend_sm.clear()

        def consume_entry(ent, jj):
            ek, ejg, eusb, eex, _, eselrz = ent
            consume(ek, ejg, jj, eusb[jj], eex, eselrz[0], dve_halves(ek, ejg))

        for k, jg in groups:
            if jg == 0:
                s_t = psum_s.tile([BL, 2 * OD], F32, name=f"sacc_{k}", tag="sacc")
                s_ps[k] = s_t[:8, :]
            ex = small.tile([128, G, 2, O], BF16, tag="ex")
            se = small.tile([128, G * 2], F32, tag="se")
            ent = [k, jg, [], ex, se, [None]]
            for jj in range(G):
                j2 = jg * G + jj
                if k == 1 and j2 in prefix_usb:
                    u_sb = prefix_usb.pop(j2)
                else:
                    u_sb = produce_u(k, j2)
                ent[2].append(u_sb)
                if vexp_ready[k]:
                    produce_q(k, j2, u_sb, ex, se)
                else:
                    pend_q.append((k, j2, u_sb, ex, se))
                if k == 1 and jg == 0 and jj == 2:
                    end_of_pass0()
                    vexp_ready[1] = True
                    flush_pend()
                if len(pipeline) >= DEPTH:
                    old = pipeline[0]
                    consume_entry(old, jj)
                    if jj == G - 1:
                        pipeline.pop(0)
                        if old[1] == NG - 1:
                            end_of_pass(old[0])
                            vexp_ready[old[0] + 1] = True
                            flush_pend()
            if vexp_ready[k]:
                ent[5][0] = smalls(k, jg, ex, se)
            else:
                pend_sm.append(ent)
            pipeline.append(ent)
        while pipeline:
            old = pipeline.pop(0)
            for jj in range(G):
                consume_entry(old, jj)
            if old[1] == NG - 1:
                end_of_pass(old[0])
                if old[0] == 1:
                    vexp_ready[2] = True
                    flush_pend()

    nc.compile()
    return nc


_nc_cache = {}


def _get_nc():
    if "nc" not in _nc_cache:
        _nc_cache["nc"] = build_nc()
    return _nc_cache["nc"]


def _prep_host(x, W):
    """Build the per-core input maps (numpy only)."""
    # W16[j2][(n,i)][(o,d)] = W[16*j2+n, o, i, d]
    W16 = np.ascontiguousarray(
        W.reshape(J2, 16, O, I, D)
        .transpose(0, 1, 3, 2, 4)
        .reshape(J2, 128, OD)
        .transpose(1, 0, 2)
    ).astype(_BF)
    ones_bd = np.zeros((128, 8), dtype=_BF)
    for p in range(128):
        ones_bd[p, p % 8] = 1.0
    sel16_h = np.zeros((16, 2, 128), dtype=_BF)
    for h in range(2):
        for m in range(128):
            sel16_h[h * 8 + (m % 8), h, m] = 1.0
    in_maps = []
    for c in range(CORES):
        xl = x[c * BL : (c + 1) * BL]  # [16, 2048, 8]
        T = xl.reshape(BL, J2, 16, I).transpose(1, 2, 3, 0)  # [j2, n, i, b]
        xt = np.ascontiguousarray(
            (T / 32.0).reshape(J2, 128, BL).transpose(1, 0, 2)
        ).astype(_BF)
        xbd = np.zeros((J2, 128, 2, 128), dtype=np.float32)
        for n in range(16):
            xbd[:, n * 8 : (n + 1) * 8, 0, n * 8 : (n + 1) * 8] = T[:, n, :, 0:8]
            xbd[:, n * 8 : (n + 1) * 8, 1, n * 8 : (n + 1) * 8] = T[:, n, :, 8:16]
        in_maps.append(
            {
                "w": W16,
                "xt": xt,
                "xbd": xbd.reshape(J2, 128, 256).astype(_BF),
                "ones": ones_bd,
                "sel16": sel16_h,
            }
        )
    return in_maps


TRACE = False
_last = {}


def kernel(x: np.ndarray, W: np.ndarray) -> np.ndarray:
    nc = _get_nc()
    in_maps = _prep_host(
        np.asarray(x, dtype=np.float32), np.asarray(W, dtype=np.float32)
    )
    res = run_bass_kernel_spmd(
        nc, in_maps, core_ids=list(range(CORES)), trace=TRACE
    )
    _last["res"] = res
    outs = [r["out"].reshape(BL, O, D) for r in res.results]
    return np.concatenate(outs, axis=0).astype(np.float32)


if __name__ == "__main__":
    rng = np.random.default_rng(0)
    x = rng.standard_normal((B, N, I), dtype=np.float32)
    W = rng.standard_normal((N, O, I, D), dtype=np.float32)
    v = kernel(x, W)
    print(v.shape, v.dtype, float(np.abs(v).mean()))


# revision 3
# speedup vs baseline: 1.0136x; 1.0136x over previous
"""CapsuleLayer (dynamic routing) Trainium2 kernel.

x: [128, 2048, 8] f32, W: [2048, 32, 8, 16] f32 -> v: [128, 32, 16] f32

Sharding: batch B=128 split across 8 cores (16 each), W replicated (96 of
128 j2-tiles resident in SBUF bf16, the rest streamed per use).  Per core,
per routing pass, u_hat tiles ([128, 1024] = 16 caps x 16 batch x 512
(o,d)) are recomputed on the PE via a block-diagonal-x matmul and consumed
on-chip; u_hat never touches HBM.

Engine split per j2 (cost-model-informed):
  PE   u-matmuls + s-matmul whose stationary selrz = block-ones * (1/Z)
       folds the softmax normalization into the n-reduction
  ACT  PSUM->SBUF u cast + per-(j2,h) exp with fused accum_out Z-sums
  DVE  q = u*vexp (bf16 packed TensorTensor runs at 2x) and the d-sum as a
       2x add-tree (TensorReduce is always 1x); b-logit updates in bf16
  Pool most e = u*expb multiplies (plain TensorTensor, the only
       elementwise op the Pool/GPSIMD engine supports on real hw)

Emission is software-pipelined two groups deep: produce(group g)
interleaves with consume(group g-2) at j2 granularity so no engine's
in-order queue head-of-line blocks another stage; q/tree emission defers
across pass boundaries until the new v (vexp) has been emitted, and vexp
itself is rebuilt with K=8/16 selector matmuls instead of DMA fan-out.
"""

from contextlib import ExitStack

import numpy as np
import ml_dtypes

import concourse.bass as bass
import concourse.bacc as bacc
import concourse.tile as tile
from concourse import mybir
from concourse.bass_utils import run_bass_kernel_spmd

BF16 = mybir.dt.bfloat16
F32 = mybir.dt.float32
X = mybir.AxisListType.X
Exp = mybir.ActivationFunctionType.Exp
Copy = mybir.ActivationFunctionType.Copy

B, N, O, I, D = 128, 2048, 32, 8, 16
CORES = 8
BL = B // CORES            # 16 batch elements per core
J2 = N // 16               # 128 blocks of 16 input caps
OD = O * D                 # 512
G = 4                      # j2 group size for batched softmax
NG = J2 // G               # groups per pass
JRES = 96                  # bf16 W j2-tiles resident in SBUF

_BF = ml_dtypes.bfloat16


def _bcast_last(ap, count):
    """Append a step-0 (broadcast) innermost dim to an AP."""
    return bass.AP(tensor=ap.tensor, offset=ap.offset, ap=list(ap.ap) + [[0, count]])


def build_nc():
    nc = bacc.Bacc("TRN2", target_bir_lowering=False)

    w = nc.dram_tensor("w", [128, J2, OD], BF16, kind="ExternalInput")
    xt = nc.dram_tensor("xt", [128, J2, BL], BF16, kind="ExternalInput")
    xbd = nc.dram_tensor("xbd", [J2, 128, 2 * 128], BF16, kind="ExternalInput")
    ones = nc.dram_tensor("ones", [128, 8], BF16, kind="ExternalInput")
    sel16 = nc.dram_tensor("sel16", [16, 2, 128], BF16, kind="ExternalInput")
    out = nc.dram_tensor("out", [BL, OD], F32, kind="ExternalOutput")

    with tile.TileContext(nc) as tc, ExitStack() as ctx:
        xbdp = ctx.enter_context(tc.tile_pool(name="xbdp", bufs=8))
        wsp = ctx.enter_context(tc.tile_pool(name="wsp", bufs=4))
        const = ctx.enter_context(tc.tile_pool(name="const", bufs=1))
        biasp = ctx.enter_context(tc.tile_pool(name="biasp", bufs=1))
        vexpp = ctx.enter_context(tc.tile_pool(name="vexpp", bufs=2))
        work = ctx.enter_context(tc.tile_pool(name="work", bufs=3))
        small = ctx.enter_context(tc.tile_pool(name="small", bufs=6))
        sqp = ctx.enter_context(tc.tile_pool(name="sqp", bufs=1))
        psum_u = ctx.enter_context(tc.tile_pool(name="psum_u", bufs=3, space="PSUM"))
        psum_s = ctx.enter_context(tc.tile_pool(name="psum_s", bufs=1, space="PSUM"))

        Mult = mybir.AluOpType.mult

        ones_sb = const.tile([128, 8], BF16)
        nc.sync.dma_start(out=ones_sb[:], in_=ones[:])
        sel_sb = const.tile([16, 2, 128], BF16)
        nc.sync.dma_start(out=sel_sb[:], in_=sel16[:])
        xt_all = const.tile([128, J2, BL], BF16)
        nc.sync.dma_start(out=xt_all[:], in_=xt[:])
        NPREF = 12
        w_all = const.tile([128, JRES, OD], BF16)
        for ch in range(6):
            nc.sync.dma_start(
                out=w_all[:, ch * 16 : (ch + 1) * 16, :],
                in_=w[:][:, ch * 16 : (ch + 1) * 16, :],
            )

        def w_tile(j2):
            if j2 < JRES:
                return w_all[:, j2, :]
            wt = wsp.tile([128, OD], BF16, tag="wst")
            nc.sync.dma_start(out=wt[:], in_=w[:][:, j2, :])
            return wt[:]

        # bias logits [(n16 b8) partition, (j2, h, o)] bf16
        bias_all = biasp.tile([128, J2, 2, O], BF16)

        # prewarm ACT sqrt/exp tables so LoadActFuncSet is off the critical path
        warm = sqp.tile([1, 2], F32, tag="warm")
        nc.vector.memset(warm[:], 1.0)
        nc.scalar.sqrt(warm[:, 0:1], warm[:, 0:1])
        nc.scalar.activation(warm[:, 1:2], warm[:, 0:1], Exp)

        epsb = const.tile([128, 1], F32)
        nc.vector.memset(epsb[:], 1e-8)

        def squash(s_ap, P, v_ap):
            """v = s * |s|^2/(1+|s|^2) / sqrt(|s|^2 + 1e-8), per (b, o) over d."""
            s_sb = sqp.tile([P, OD], F32, tag="s_sb")
            nc.scalar.activation(s_sb[:], s_ap, Copy)
            ssq = sqp.tile([P, OD], F32, tag="ssq")
            nc.vector.tensor_mul(ssq[:], s_sb[:], s_sb[:])
            sq = sqp.tile([P, O], F32, tag="sq")
            nc.vector.reduce_sum(
                out=sq[:], in_=ssq[:].rearrange("p (o d) -> p o d", d=D), axis=X
            )
            rt = sqp.tile([P, O], F32, tag="rt")
            nc.scalar.activation(rt[:], sq[:], mybir.ActivationFunctionType.Sqrt, bias=epsb[:P, :])
            g = sqp.tile([P, O], F32, tag="g")
            nc.vector.scalar_tensor_tensor(
                g[:], sq[:], 1.0, rt[:], mybir.AluOpType.add, Mult
            )
            rg = sqp.tile([P, O], F32, tag="rg")
            nc.vector.reciprocal(rg[:], g[:])
            scale = sqp.tile([P, O], F32, tag="scale")
            nc.vector.tensor_mul(scale[:], sq[:], rg[:])
            nc.vector.tensor_mul(
                v_ap.rearrange("p (o d) -> p o d", d=D),
                s_sb[:].rearrange("p (o d) -> p o d", d=D),
                _bcast_last(scale[:], D),
            )

        def squash2(s_ap, P, v_ap=None):
            """squash() over [P, 2*OD] treating (h,o) as 64 capsules.

            With v_ap=None the (dead-after-reduce) ssq tile is reused as the
            f32 output and returned."""
            s_sb = sqp.tile([P, 2 * OD], F32, tag="s2_sb")
            nc.scalar.activation(s_sb[:], s_ap, Copy)
            ssq = sqp.tile([P, 2 * OD], F32, tag="s2sq")
            nc.vector.tensor_mul(ssq[:], s_sb[:], s_sb[:])
            sq = sqp.tile([P, 2 * O], F32, tag="s2q")
            nc.vector.reduce_sum(
                out=sq[:], in_=ssq[:].rearrange("p (o d) -> p o d", d=D), axis=X
            )
            rt = sqp.tile([P, 2 * O], F32, tag="s2rt")
            nc.scalar.activation(
                rt[:], sq[:], mybir.ActivationFunctionType.Sqrt, bias=epsb[:P, :]
            )
            g = sqp.tile([P, 2 * O], F32, tag="s2g")
            nc.vector.scalar_tensor_tensor(
                g[:], sq[:], 1.0, rt[:], mybir.AluOpType.add, Mult
            )
            rg = sqp.tile([P, 2 * O], F32, tag="s2rg")
            nc.vector.reciprocal(rg[:], g[:])
            scale = sqp.tile([P, 2 * O], F32, tag="s2scale")
            nc.vector.tensor_mul(scale[:], sq[:], rg[:])
            if v_ap is None:
                v_ap = ssq[:]
            nc.vector.tensor_mul(
                v_ap.rearrange("p (o d) -> p o d", d=D),
                s_sb[:].rearrange("p (o d) -> p o d", d=D),
                _bcast_last(scale[:], D),
            )
            return v_ap

        def make_vexp(vfull):
            """vfull: [16, OD] bf16 tile (v rows) -> vexp [128, 2*OD] tile.

            vexp[p=(n16 b8), h*OD + (o,d)] = v[h*8 + p%8, o, d], built with two
            selector matmuls (sel16[k,h,m] = d(k, h*8+m%8)) + one ACT cast, so
            no SP-sequencer DMA sits on the pass-boundary critical path.
            """
            vx_ps = psum_u.tile([128, 2 * OD], F32, tag="ups")
            for h in range(2):
                nc.tensor.matmul(
                    vx_ps[:, h * OD : (h + 1) * OD],
                    sel_sb[:, h, :],
                    vfull[:],
                    start=True,
                    stop=True,
                )
            vx = vexpp.tile([128, 2 * OD], BF16, tag="vexp")
            nc.scalar.activation(vx[:], vx_ps[:], Copy)
            return vx

        def produce_u(k, j2):
            xbd_t = xbdp.tile([128, 2 * 128], BF16)
            nc.sync.dma_start(out=xbd_t[:], in_=xbd[:][j2, :, :])
            u_ps = psum_u.tile([128, 2 * OD], F32, tag="ups")
            wj = w_tile(j2)
            for h in range(2):
                nc.tensor.matmul(
                    u_ps[:, h * OD : (h + 1) * OD],
                    xbd_t[:, h * 128 : (h + 1) * 128],
                    wj,
                    start=True,
                    stop=True,
                )
            u_sb = work.tile([128, 2 * OD], BF16, tag="usb", bufs=16)
            nc.scalar.activation(u_sb[:], u_ps[:], Copy)
            return u_sb

        # ---------------- pass 1 (iter 0): s0 = sum_n u_hat / 32 ----------------
        s0_t = psum_s.tile([BL, 2 * OD], F32, tag="sacc")
        s0_ps = s0_t[:, :OD]
        # Interleave the first pass-2 u-productions into the s0 chain: the
        # s0 matmuls are gated on W-chunk DMA arrival, so PE/ACT have slack
        # to pre-produce u tiles; Pool then has a deep backlog at v0-time.
        prefix_usb = {}
        for j2 in range(J2):
            nc.tensor.matmul(
                s0_ps,
                xt_all[:, j2, :],
                w_tile(j2),
                start=(j2 == 0),
                stop=(j2 == J2 - 1),
            )
            if j2 % 8 == 7 and len(prefix_usb) < NPREF:
                pj = len(prefix_usb)
                prefix_usb[pj] = produce_u(1, pj)
        vexp = [None]

        def end_of_pass0():
            v_full1 = vexpp.tile([BL, OD], BF16, tag="vfull")
            squash(s0_ps, BL, v_full1[:])
            vexp[0] = make_vexp(v_full1)

        # ---------------- passes 2, 3 (iters 1, 2): software pipeline ----------
        # produce_u(k, j2): xbd DMA + 2 u-matmuls + ACT cast        (no v dep)
        # produce_q(k, j2): q = u*vexp, 2x add-tree -> bias logits  (needs vexp)
        # smalls(k, jg):    exp, Z-reduce, 1/Z, selrz               (needs trees)
        # consume(k, jg, jj): e = u*expb (Pool/DVE), s-matmuls      (needs smalls)

        s_ps = {}
        pend_badd = []

        def produce_q(k, j2, u_sb, ex, se):
            q = work.tile([128, 2 * OD], BF16, tag="q", bufs=2)
            nc.vector.tensor_mul(q[:], u_sb[:], vexp[0][:])
            qv = q[:].rearrange("p (ho d) -> p ho d", d=D)
            t1 = work.tile([128, 64, 8], BF16, tag="t1", bufs=1)
            nc.vector.tensor_add(t1[:], qv[:, :, 0:8], qv[:, :, 8:16])
            t2 = work.tile([128, 64, 4], BF16, tag="t2", bufs=1)
            nc.vector.tensor_add(t2[:], t1[:, :, 0:4], t1[:, :, 4:8])
            t3 = work.tile([128, 64, 2], BF16, tag="t3", bufs=1)
            nc.vector.tensor_add(t3[:], t2[:, :, 0:2], t2[:, :, 2:4])
            bias_slice = bias_all[:, j2, :, :]  # [128, 2, O]
            if k == 1:
                nc.vector.tensor_add(
                    bias_slice.rearrange("p h o -> p (h o)"),
                    t3[:, :, 0],
                    t3[:, :, 1],
                )
            else:
                a2 = small.tile([128, 64], BF16, name="a2", tag="a2")
                nc.vector.tensor_add(a2[:], t3[:, :, 0], t3[:, :, 1])
                nc.vector.tensor_add(
                    bias_slice.rearrange("p h o -> p (h o)"),
                    bias_slice.rearrange("p h o -> p (h o)"),
                    a2[:],
                )
            jj = j2 % G
            for h in range(2):
                nc.scalar.activation(
                    ex[:, jj, h, :],
                    bias_slice[:, h, :],
                    Exp,
                    accum_out=se[:, 2 * jj + h : 2 * jj + h + 1],
                )


        def smalls(k, jg, ex, se):
            rse = small.tile([128, G * 2], F32, tag="rse")
            nc.vector.reciprocal(rse[:], se[:])
            rse = rse[:].rearrange("p (g h) -> p g h", h=2)
            selrz = small.tile([128, G, 2, 8], BF16, tag="selrz")
            oap = ones_sb[:]
            ones_b = bass.AP(
                tensor=oap.tensor,
                offset=oap.offset,
                ap=[list(oap.ap)[0], [0, G], [0, 2], list(oap.ap)[1]],
            )
            nc.vector.tensor_mul(selrz[:], ones_b, _bcast_last(rse, 8))
            return selrz

        def consume(k, jg, jj, u_sb, ex, selrz, n_dve):
            j2 = jg * G + jj
            e_t = work.tile([128, 2 * OD], BF16, tag="et", bufs=6)
            for h in range(2):
                eng = nc.vector if (2 * jj + h) >= 8 - n_dve else nc.gpsimd
                eng.tensor_mul(
                    e_t[:, h * OD : (h + 1) * OD].rearrange("p (o d) -> p o d", d=D),
                    u_sb[:, h * OD : (h + 1) * OD].rearrange("p (o d) -> p o d", d=D),
                    _bcast_last(ex[:, jj, h, :], D),
                )
            for h in range(2):
                nc.tensor.matmul(
                    s_ps[k][:, h * OD : (h + 1) * OD],
                    selrz[:, jj, h, :],
                    e_t[:, h * OD : (h + 1) * OD],
                    start=(j2 == 0),
                    stop=(j2 == J2 - 1),
                )

        def make_vexp8(vtmp):
            """vtmp: [8, 2*OD] bf16 (v rows as h-halves) -> vexp tile.

            vexp[p, h*OD+(o,d)] = vtmp[p%8, h*OD+(o,d)] via K=8 selector
            matmuls; no staging DMA on the pass-boundary critical path.
            """
            vx_ps = psum_u.tile([128, 2 * OD], F32, tag="ups")
            sel8 = sel_sb[0:8, 0, :]
            for h in range(2):
                nc.tensor.matmul(
                    vx_ps[:, h * OD : (h + 1) * OD],
                    sel8,
                    vtmp[:, h * OD : (h + 1) * OD],
                    start=True,
                    stop=True,
                )
            vx = vexpp.tile([128, 2 * OD], BF16, tag="vexp")
            nc.scalar.activation(vx[:], vx_ps[:], Copy)
            return vx

        def end_of_pass(k):
            """squash(s) -> v; rebuild vexp (k<2) or write output (k=2)."""
            sp = s_ps[k]
            if k == 1:
                vtmp = sqp.tile([8, 2 * OD], BF16, tag="vtmp")
                squash2(sp, 8, vtmp[:])
                vexp[0] = make_vexp8(vtmp)
            else:
                vv = squash2(sp, 8, None)
                for h in range(2):
                    nc.sync.dma_start(
                        out=out[:][h * 8 : (h + 1) * 8, :],
                        in_=vv[:, h * OD : (h + 1) * OD],
                    )

        # DVE e-share pattern: ~1.5 of 8 e-halves per group go to DVE
        def dve_halves(k, jg):
            return 2

        groups = [(k, jg) for k in (1, 2) for jg in range(NG)]
        DEPTH = 2
        pipeline = []  # entries: [k, jg, usb, ex, se, selrz_holder]
        pend_q = []    # produce_q deferred until the pass's vexp exists
        pend_sm = []   # entries with smalls deferred for the same reason
        vexp_ready = {1: False, 2: False}

        def flush_pend():
            # emit per group in consume order: q/trees then that group's
            # smalls, so the first group's e-chain unblocks ASAP
            sm_left = list(pend_sm)
            by_g = {}
            for item in pend_q:
                by_g.setdefault((item[0], item[1] // G), []).append(item)
            for key in sorted(by_g):
                for qk, qj2, qusb, qex, qse in by_g[key]:
                    produce_q(qk, qj2, qusb, qex, qse)
                for ent in list(sm_left):
                    if (ent[0], ent[1]) == key:
                        ent[5][0] = smalls(ent[0], ent[1], ent[3], ent[4])
                        sm_left.remove(ent)
            for ent in sm_left:
                ent[5][0] = smalls(ent[0], ent[1], ent[3], ent[4])
            pend_q.clear()
            pend_sm.clear()

        def consume_entry(ent, jj):
            ek, ejg, eusb, eex, _, eselrz = ent
            consume(ek, ejg, jj, eusb[jj], eex, eselrz[0], dve_halves(ek, ejg))

        for k, jg in groups:
            if jg == 0:
                s_t = psum_s.tile([BL, 2 * OD], F32, name=f"sacc_{k}", tag="sacc")
                s_ps[k] = s_t[:8, :]
            ex = small.tile([128, G, 2, O], BF16, tag="ex")
            se = small.tile([128, G * 2], F32, tag="se")
            ent = [k, jg, [], ex, se, [None]]
            for jj in range(G):
                j2 = jg * G + jj
                if k == 1 and j2 in prefix_usb:
                    u_sb = prefix_usb.pop(j2)
                else:
                    u_sb = produce_u(k, j2)
                ent[2].append(u_sb)
                if vexp_ready[k]:
                    produce_q(k, j2, u_sb, ex, se)
                else:
                    pend_q.append((k, j2, u_sb, ex, se))
                if k == 1 and jg == 1 and jj == 3:
                    end_of_pass0()
                    vexp_ready[1] = True
                    flush_pend()
                if len(pipeline) >= DEPTH:
                    old = pipeline[0]
                    consume_entry(old, jj)
                    if jj == G - 1:
                        pipeline.pop(0)
                        if old[1] == NG - 1:
                            end_of_pass(old[0])
                            vexp_ready[old[0] + 1] = True
                            flush_pend()
            if vexp_ready[k]:
                ent[5][0] = smalls(k, jg, ex, se)
            else:
                pend_sm.append(ent)
            pipeline.append(ent)
        while pipeline:
            old = pipeline.pop(0)
            for jj in range(G):
                consume_entry(old, jj)
            if old[1] == NG - 1:
                end_of_pass(old[0])
                if old[0] == 1:
                    vexp_ready[2] = True
                    flush_pend()

    nc.compile()
    return nc


_nc_cache = {}


def _get_nc():
    if "nc" not in _nc_cache:
        _nc_cache["nc"] = build_nc()
    return _nc_cache["nc"]


def _prep_host(x, W):
    """Build the per-core input maps (numpy only)."""
    # W16[j2][(n,i)][(o,d)] = W[16*j2+n, o, i, d]
    W16 = np.ascontiguousarray(
        W.reshape(J2, 16, O, I, D)
        .transpose(0, 1, 3, 2, 4)
        .reshape(J2, 128, OD)
        .transpose(1, 0, 2)
    ).astype(_BF)
    ones_bd = np.zeros((128, 8), dtype=_BF)
    for p in range(128):
        ones_bd[p, p % 8] = 1.0
    sel16_h = np.zeros((16, 2, 128), dtype=_BF)
    for h in range(2):
        for m in range(128):
            sel16_h[h * 8 + (m % 8), h, m] = 1.0
    in_maps = []
    for c in range(CORES):
        xl = x[c * BL : (c + 1) * BL]  # [16, 2048, 8]
        T = xl.reshape(BL, J2, 16, I).transpose(1, 2, 3, 0)  # [j2, n, i, b]
        xt = np.ascontiguousarray(
            (T / 32.0).reshape(J2, 128, BL).transpose(1, 0, 2)
        ).astype(_BF)
        xbd = np.zeros((J2, 128, 2, 128), dtype=np.float32)
        for n in range(16):
            xbd[:, n * 8 : (n + 1) * 8, 0, n * 8 : (n + 1) * 8] = T[:, n, :, 0:8]
            xbd[:, n * 8 : (n + 1) * 8, 1, n * 8 : (n + 1) * 8] = T[:, n, :, 8:16]
        in_maps.append(
            {
                "w": W16,
                "xt": xt,
                "xbd": xbd.reshape(J2, 128, 256).astype(_BF),
                "ones": ones_bd,
                "sel16": sel16_h,
            }
        )
    return in_maps


TRACE = False
_last = {}


def kernel(x: np.ndarray, W: np.ndarray) -> np.ndarray:
    nc = _get_nc()
    in_maps = _prep_host(
        np.asarray(x, dtype=np.float32), np.asarray(W, dtype=np.float32)
    )
    res = run_bass_kernel_spmd(
        nc, in_maps, core_ids=list(range(CORES)), trace=TRACE
    )
    _last["res"] = res
    outs = [r["out"].reshape(BL, O, D) for r in res.results]
    return np.concatenate(outs, axis=0).astype(np.float32)


if __name__ == "__main__":
    rng = np.random.default_rng(0)
    x = rng.standard_normal((B, N, I), dtype=np.float32)
    W = rng.standard_normal((N, O, I, D), dtype=np.float32)
    v = kernel(x, W)
    print(v.shape, v.dtype, float(np.abs(v).mean()))
